# revision 14
# baseline (speedup 1.0000x reference)
"""Trainium2 Bass kernel for GTPCA topk_masking layer.

Computation (see reference):
  wn     = w / sqrt(sum(w^2)/n),  n = 128*128
  scores = valid_xcorr2d(inputs, wn) / n          -> (B, 113, 113)
  idx    = argmax |scores| (flat, first occurrence)
  out    = scores[idx] * wn placed as a 16x16 patch at idx, zeros elsewhere

Device strategy (pure data parallel over 8 cores, 512 images each):
  - Inputs and the 128x16x113 row-Toeplitz weight tensor are quantized to
    fp8e4 (e4m3) on the host.  The 2D correlation runs on the tensor engine
    as 8 DoubleRow matmuls per 4-image bank: each DoubleRow matmul contracts
    over 2x128 (a (q, q+1) pair of kernel columns) and streams 2 output
    columns per cycle -> 4x fewer PE cycles than the fp32r baseline.
  - The 4 images of a bank are interleaved along the innermost SBUF axis
    ([H, W_PAD, 4]) so the (q, q+1) k-tile pair is expressible as a 3-dim
    access pattern: [[part,128],[4,2],[1,456]] (k-tile stride = one w step).
  - PSUM holds scores*n as [113, w*4+img]; a fused DVE reduce with
    apply_absolute_value takes the per-row abs-max over w per image.
  - Only the per-row abs-max [113, 512] leaves the device.  The host finds
    candidate rows within CAND_TOL of each image's global max (covers fp8
    quantization noise), rescores those rows exactly in fp64, picks the true
    argmax, and scatters smax*wn patches into the output.
"""

import sys

import numpy as np

if "/opt/trn_rl_repo" not in sys.path:
    sys.path.insert(0, "/opt/trn_rl_repo")

import ml_dtypes

FP8 = ml_dtypes.float8_e4m3

N_CORES = 8
B = 4096
H = W = 128
KH = KW = 16
SH = SW = H - KH + 1  # 113
SW_PAD = 114  # per-image output columns in PSUM (needs q+1+113 <= W_PAD)
W_PAD = 130  # padded image width
N_ELEM = H * W  # 16384
PER_CORE = B // N_CORES  # 512
CHUNK = 64  # images per DMA (big transfers amortize per-DMA dispatch latency)
BANK = 4  # images per PSUM bank, interleaved innermost (4*114 = 456 <= 512)
CAND_TOL = 1e-1  # candidate-row gate vs device global max (covers fp8 noise)
SH_PAD = 128  # ttoe inner-dim padding (ldweights ISA needs 128-elem k-tile stride)
DMA_QUEUES = 1  # chunk DMAs on the SP queue (dual-queue A/B measured slower)


def _build_nc(n_imgs: int, repeat: int = 1):
    from contextlib import ExitStack

    import concourse.bacc as bacc
    import concourse.mybir as mybir
    import concourse.tile as tile

    f32 = mybir.dt.float32
    f8 = mybir.dt.float8e4

    n_banks = n_imgs // BANK
    banks_per_chunk = CHUNK // BANK
    n_chunks = n_imgs // CHUNK

    nc = bacc.Bacc("TRN2", target_bir_lowering=False)
    # imgs_d[h, bank, w, lane] = image_{bank*4+lane}[h, w]
    imgs_d = nc.dram_tensor("imgs", [H, n_banks, W_PAD, BANK], f8, kind="ExternalInput")
    ttoe_d = nc.dram_tensor("ttoe", [H, KW, SH_PAD], f8, kind="ExternalInput")
    rm_d = nc.dram_tensor("rowmax", [SH, n_imgs], f32, kind="ExternalOutput")

    with ExitStack() as ctx:
        tc = ctx.enter_context(tile.TileContext(nc))
        consts = ctx.enter_context(tc.tile_pool(name="consts", bufs=1))
        imgp = ctx.enter_context(tc.tile_pool(name="imgp", bufs=3))
        accp = ctx.enter_context(tc.tile_pool(name="accp", bufs=8, space="PSUM"))
        stage = ctx.enter_context(tc.tile_pool(name="stage", bufs=1))

        ttoe_t = consts.tile([H, KW, SH_PAD], f8)
        nc.sync.dma_start(ttoe_t[:], ttoe_d[:])
        rm_all = stage.tile([SH, n_imgs], f32)

        for _rep in range(repeat):
          for ch in range(n_chunks):
            img_t = imgp.tile([H, banks_per_chunk, W_PAD, BANK], f8)
            dma_eng = nc.sync if (DMA_QUEUES == 1 or ch % 2 == 0) else nc.scalar
            dma_eng.dma_start(
                img_t[:],
                imgs_d[:, ch * banks_per_chunk : (ch + 1) * banks_per_chunk, :, :],
            )

            for bk in range(banks_per_chunk):
                psum = accp.tile([SH, SW_PAD * BANK], f32, name="acc", tag="acc")
                for qp in range(KW // 2):
                    q = 2 * qp
                    lhs = ttoe_t[:, q : q + 2, 0:SH]  # [128, 2, 113]
                    # moving AP [128, 2, 456]: k-tile = (q, q+1) via stride-4
                    # (one w step in the interleaved layout); inner run covers
                    # (w - q) * 4 + lane for 114 w's x 4 lanes.
                    rhs = img_t[:, bk, q : q + SW_PAD, :].copy()
                    rhs.ap = rhs.ap[:1] + [[BANK, 2], [1, SW_PAD * BANK]]
                    nc.tensor.matmul(
                        psum[:],
                        lhs,
                        rhs,
                        start=(qp == 0),
                        stop=(qp == KW // 2 - 1),
                        perf_mode=mybir.MatmulPerfMode.DoubleRow,
                        skip_group_check=True,
                    )
                base = ch * CHUNK + bk * BANK
                # view PSUM [113, 456] as [113, lane(4), w(113, stride 4)] and
                # abs-max-reduce over w (innermost, strided)
                src = psum[:].copy()
                src.ap = src.ap[:1] + [[1, BANK], [BANK, SW]]
                nc.vector.tensor_reduce(
                    rm_all[:, base : base + BANK],
                    src,
                    axis=mybir.AxisListType.X,
                    op=mybir.AluOpType.max,
                    apply_absolute_value=True,
                )

        nc.sync.dma_start(rm_d[:], rm_all[:])

    nc.compile()
    return nc


_NC_CACHE: dict = {}


def _get_nc(n_imgs: int):
    if n_imgs not in _NC_CACHE:
        _NC_CACHE[n_imgs] = _build_nc(n_imgs)
    return _NC_CACHE[n_imgs]


def _weights_f32(w: np.ndarray) -> np.ndarray:
    w32 = np.asarray(w, dtype=np.float32)
    ss = np.sum(w32 * w32, dtype=np.float32)
    denom = np.sqrt(ss / np.float32(N_ELEM))
    return (w32 / denom).astype(np.float32)


def _toeplitz8(wn: np.ndarray) -> np.ndarray:
    wn8 = wn.astype(FP8).astype(np.float32)
    ttoe = np.zeros((H, KW, SH_PAD), dtype=np.float32)
    for i in range(SH):
        ttoe[i : i + KH, :, i] = wn8
    return ttoe.astype(FP8)


def _pack_imgs(inputs_np: np.ndarray) -> np.ndarray:
    """[B, H, W] f32 -> [H, B//4, W_PAD, 4] fp8, images interleaved by 4."""
    nb = inputs_np.shape[0]
    x8 = inputs_np.astype(FP8)  # [B, H, W]
    arr = np.zeros((H, nb // BANK, W_PAD, BANK), dtype=FP8)
    # (h, bank, w, lane) = image_{bank*4+lane}[h, w]
    arr[:, :, :W, :] = x8.reshape(nb // BANK, BANK, H, W).transpose(2, 0, 3, 1)
    return arr


def _run_device(imgs_packed: np.ndarray, ttoe8: np.ndarray, trace: bool = False):
    from concourse.bass_utils import run_bass_kernel_spmd

    nc = _get_nc(PER_CORE)
    nbanks_core = PER_CORE // BANK
    in_maps = []
    for c in range(N_CORES):
        shard = np.ascontiguousarray(
            imgs_packed[:, c * nbanks_core : (c + 1) * nbanks_core, :, :]
        )
        in_maps.append({"imgs": shard, "ttoe": ttoe8})
    res = run_bass_kernel_spmd(
        nc, in_maps, core_ids=list(range(N_CORES)), trace=trace
    )
    rm = np.concatenate([r["rowmax"] for r in res.results], axis=1)  # [113, B]
    return rm, res


def _finalize(inputs_np: np.ndarray, wn: np.ndarray, rm: np.ndarray) -> np.ndarray:
    """Host: candidate rows -> exact rescore -> argmax -> patch scatter."""
    nb = rm.shape[1]
    gm = rm.max(axis=0)  # [nb] device global abs-max per image
    thr = gm * (1.0 - CAND_TOL)
    cb, ci = np.nonzero((rm >= thr[None, :]).T)  # image ids, candidate rows

    # exact scores for each candidate row, fp64, via one dgemm + shift-add
    row_idx = ci[:, None] + np.arange(KH)[None, :]  # [C, 16]
    strips = inputs_np[cb[:, None], row_idx, :].astype(np.float64)  # [C, 16, 128]
    wn64 = wn.astype(np.float64)
    n_cand = len(cb)
    # A[c, w, q] = sum_p strips[c, p, w] * wn[p, q]
    A = np.tensordot(strips, wn64, axes=([1], [0]))  # [C, 128, 16]
    scores = np.zeros((n_cand, SW), dtype=np.float64)
    for q in range(KW):
        scores += A[:, q : q + SW, q]
    scores /= float(N_ELEM)

    # per image: among candidate rows pick max |score|, ties -> lowest flat idx
    flat = ci[:, None].astype(np.int64) * SW + np.arange(SW)[None, :]
    abss = np.abs(scores)
    best_val = np.zeros(nb, dtype=np.float64)
    best_flat = np.zeros(nb, dtype=np.int64)
    best_abs = np.full(nb, -1.0, dtype=np.float64)
    # reduce per candidate-row first
    j_best = np.argmax(abss, axis=1)  # first occurrence within row
    r_abs = abss[np.arange(n_cand), j_best]
    r_val = scores[np.arange(n_cand), j_best]
    r_flat = flat[np.arange(n_cand), j_best]
    # then reduce across rows of the same image (first occurrence on exact ties)
    order = np.lexsort((r_flat, -r_abs, cb))  # grouped by image
    cb_o = cb[order]
    first = np.unique(cb_o, return_index=True)[1]
    sel = order[first]
    img_ids = cb[sel]
    best_val[img_ids] = r_val[sel]
    best_flat[img_ids] = r_flat[sel]
    best_abs[img_ids] = r_abs[sel]
    assert np.all(best_abs >= 0.0), "some image had no candidate rows"

    rows = (best_flat // SW).astype(np.int64)
    cols = (best_flat % SW).astype(np.int64)
    vals = best_val.astype(np.float32)

    out = np.zeros((nb, H, W), dtype=np.float32)
    patches = vals[:, None, None] * wn[None, :, :]  # [nb, 16, 16] f32
    bidx = np.arange(nb)[:, None, None]
    ridx = rows[:, None, None] + np.arange(KH)[None, :, None]
    cidx = cols[:, None, None] + np.arange(KW)[None, None, :]
    out[bidx, ridx, cidx] = patches
    return out


def kernel(inputs: np.ndarray, w: np.ndarray) -> np.ndarray:
    inputs_np = np.ascontiguousarray(np.asarray(inputs, dtype=np.float32))
    wn = _weights_f32(w)
    ttoe8 = _toeplitz8(wn)
    imgs_packed = _pack_imgs(inputs_np)
    rm, _ = _run_device(imgs_packed, ttoe8)
    return _finalize(inputs_np, wn, rm)
